# revision 51
# baseline (speedup 1.0000x reference)
"""Trainium2 Bass kernel for MEGA MultiHeadEMA-style BaseMovingLayer.

Computes, for x[B, D, L] with per-channel EMA params:
    p = sigmoid(delta)*sigmoid(alpha); q = 1-p
    k[d, l] = sum_n (p*beta*gamma*scale)[d,n] * q[d,n]^l
    out = causal_conv(x, k) + x * omega[:, None]

Strategy: shard D=1024 across 8 cores (128 channels/core = 128 SBUF
partitions). Per channel the causal conv is decomposed into chunks of
C=128: intra-chunk via a per-channel Toeplitz matmul (the omega residual
is folded into the k[0] tap), inter-chunk via the EMA state recurrence
at chunk granularity (one tensor_tensor_scan per 4-channel group, with
segment resets encoded as zeros in the multiplier tensor) plus a rank-N
correction matmul. x is transposed and cast to fp16 on the host so every
device DMA is a plain contiguous copy; all matmul operands are fp16,
accumulation is fp32 in PSUM; the output is stored fp16 in a layout that
gives 2KB contiguous runs per partition and is upcast/reassembled on the
host.
"""
import sys
import os
import numpy as np

sys.path.insert(0, "/opt/trn_rl_repo")

B, D, L, N = 4, 1024, 4096, 16
NCORES = 8
DLOC = D // NCORES          # 128 channels per core
C = 128                     # chunk length
NCH = L // C                # 32 chunks
NG = DLOC // 4              # 32 groups of 4 channels
SCALE = 1.0 / np.sqrt(N)

_cache = {}


def _build_program(repeat=1):
    import concourse.bacc as bacc
    import concourse.tile as tile
    import concourse.mybir as mybir

    f16 = mybir.dt.float16
    f32 = mybir.dt.float32
    nc = bacc.Bacc("TRN2", target_bir_lowering=False, debug=False,
                   num_devices=NCORES)

    xr_d = nc.dram_tensor("xr", [C, DLOC * B * NCH], f16,
                          kind="ExternalInput").ap()
    GW = 16
    atg_d = nc.dram_tensor("atg", [(DLOC // 8) * C, 8 * (C + GW)], f16,
                           kind="ExternalInput").ap()
    f8 = mybir.dt.float8e4
    e_d = nc.dram_tensor("et", [128, NG * C], f8, kind="ExternalInput").ap()
    qc_d = nc.dram_tensor("qcw", [128, NG * C], f16,
                          kind="ExternalInput").ap()
    # fp16 out, laid out so each partition (b, c) writes one contiguous
    # 2KB run per 8-channel block; the host reassembles.
    out_d = nc.dram_tensor("out", [B * NCH, DLOC // 8, 8, C], f16,
                           kind="ExternalOutput").ap()

    with tile.TileContext(nc) as tc:
        with (
            tc.tile_pool(name="xt", bufs=1) as xt_pool,
            tc.tile_pool(name="cst", bufs=1) as cst_pool,
            tc.tile_pool(name="atg", bufs=16) as atg_pool,
            tc.tile_pool(name="spsum", bufs=2, space="PSUM") as s_pool,
            tc.tile_pool(name="opsum", bufs=3, space="PSUM") as o_pool,
            tc.tile_pool(name="h", bufs=4) as h_pool,
            tc.tile_pool(name="osb", bufs=8) as ob_pool,
        ):
            for _rep in range(repeat):
                atg_tiles = {}

                def ensure_atg(m):
                    # one tile covers 8 channels (= groups 2m, 2m+1)
                    if m in atg_tiles or m >= DLOC // 8:
                        return atg_tiles.get(m)
                    t = atg_pool.tile([128, 8 * (C + GW)], f16, tag="atg")
                    nc.sync.dma_start(t[:], atg_d[m * C:(m + 1) * C, :])
                    atg_tiles[m] = t
                    return t

                # Critical path: x chunk 0 + atg0 gate the first S-matmul,
                # qc gates the first scan, e gates the first MM2. x streams
                # on the Activation HWDGE queue, params on the SP queue.
                x_all = xt_pool.tile([128, DLOC * B * NCH], f16, tag="xall")
                e_sb = cst_pool.tile([128, NG * C], f8, tag="e")
                qc_sb = cst_pool.tile([128, NG * C], f16, tag="qc")
                xchunk = 2048
                nxc = DLOC * B * NCH // xchunk

                def load_x(i):
                    nc.sync.dma_start(
                        x_all[:, i * xchunk:(i + 1) * xchunk],
                        xr_d[:, i * xchunk:(i + 1) * xchunk],
                    )

                # All inputs on the SP queue in the exact order compute
                # consumes them; the ACT queue carries only output stores,
                # so the DMA engines drain outputs opportunistically.
                qq = NG * C // 4

                def load_sliver(m):
                    nc.sync.dma_start(qc_sb[:, m * qq:(m + 1) * qq],
                                      qc_d[:, m * qq:(m + 1) * qq])
                    nc.sync.dma_start(e_sb[:, m * qq:(m + 1) * qq],
                                      e_d[:, m * qq:(m + 1) * qq])

                load_x(0)
                ensure_atg(0)
                load_sliver(0)
                ensure_atg(1)
                load_x(1)
                ensure_atg(2)
                ensure_atg(3)
                load_x(2)
                ensure_atg(4)
                ensure_atg(5)
                load_sliver(1)
                load_x(3)
                ensure_atg(6)
                ensure_atg(7)
                load_x(4)
                ensure_atg(8)
                ensure_atg(9)
                load_sliver(2)
                load_x(5)
                ensure_atg(10)
                ensure_atg(11)
                load_x(6)
                ensure_atg(12)
                ensure_atg(13)
                load_sliver(3)
                load_x(7)
                ensure_atg(14)
                ensure_atg(15)

                h_tiles = {}
                osb_tiles = {}

                def emit_front(g):
                    # S matmuls + chunk-state scan for group g -> H tile
                    atg_t = ensure_atg(g // 2)
                    ensure_atg(g // 2 + 1)
                    s_q = s_pool.tile([128, 128], f32, tag="s")
                    nc.vector.memset(s_q[:], 0.0)
                    for j in range(4):
                        d = 4 * g + j
                        dd = d % 8
                        nc.tensor.matmul(
                            s_q[32 * j:32 * j + 16, :],
                            lhsT=atg_t[:, dd * (C + GW) + C:
                                       dd * (C + GW) + C + GW],
                            rhs=x_all[:, d * 128:(d + 1) * 128],
                            start=True, stop=True,
                            tile_position=(0, 32 * j),
                        )
                    # H[c] = qC*H[c-1] + S[c]; multiplier has zeros at each
                    # b's c=0 so the recurrence resets per batch element.
                    # Output written shifted by one (-> Hprev); the b-boundary
                    # cols (0/32/64/96) are re-zeroed afterwards (H[b,31] is
                    # never used).
                    h_t = h_pool.tile([128, 132], f16, tag="h")
                    nc.vector.tensor_tensor_scan(
                        out=h_t[:, 1:129],
                        data0=qc_sb[:, g * C:(g + 1) * C],
                        data1=s_q[:, :],
                        initial=0.0,
                        op0=mybir.AluOpType.mult,
                        op1=mybir.AluOpType.add,
                    )
                    nc.gpsimd.memset(
                        h_t[:, 0:128].rearrange("p (b c) -> p b c",
                                                c=32)[:, :, 0:1], 0.0)
                    h_tiles[g] = h_t

                def emit_back(g):
                    # output matmuls + evacuation + store for group g
                    atg_t = ensure_atg(g // 2)
                    h_t = h_tiles.pop(g)
                    blk = g // 2
                    if g % 2 == 0:
                        osb_new = ob_pool.tile([128, 8 * C], f16, tag="osb")
                        osb_tiles[blk] = osb_new
                    osb = osb_tiles[blk]
                    for pair in range(2):
                        o_p = o_pool.tile([128, 1024], f32, tag="o")
                        for j2 in range(2):
                            j = 2 * pair + j2
                            d = 4 * g + j
                            dd = d % 8
                            col0 = j2 * 512
                            nc.tensor.matmul(
                                o_p[:, col0:col0 + 128],
                                lhsT=x_all[:, d * 128:(d + 1) * 128],
                                rhs=atg_t[:, dd * (C + GW):dd * (C + GW) + C],
                                start=True, stop=False,
                            )
                            nc.tensor.matmul(
                                o_p[:, col0:col0 + 128],
                                lhsT=h_t[32 * j:32 * j + 16, 0:128],
                                rhs=e_sb[32 * j:32 * j + 16,
                                         g * C:(g + 1) * C],
                                start=False, stop=True,
                                tile_position=(32 * j, 0),
                            )
                        o_v = o_p[:].rearrange("p (dd z) -> p dd z",
                                               z=512)[:, :, 0:128]
                        dst = osb[:, ((g % 2) * 2 + pair) * 256:
                                  ((g % 2) * 2 + pair + 1) * 256]
                        if pair == 0:
                            nc.vector.tensor_copy(dst, o_v)
                        else:
                            nc.scalar.copy(dst, o_v)
                    if g % 2 == 1:
                        nc.scalar.dma_start(
                            out=out_d[:, blk, :, :],
                            in_=osb[:].rearrange("p (dd t) -> p dd t", t=C),
                        )
                        del osb_tiles[blk]

                DEPTH = 2
                for g in range(NG + DEPTH):
                    if g < NG:
                        emit_front(g)
                    if g >= DEPTH:
                        emit_back(g - DEPTH)

    nc.compile()
    return nc


def _prep_params(delta, alpha, beta, gamma, omega):
    """Host-side derivation of the small conv-kernel operand tensors."""
    delta = delta[..., 0].astype(np.float64)
    alpha = alpha[..., 0].astype(np.float64)
    beta = beta[..., 0].astype(np.float64)
    gamma = gamma.astype(np.float64)
    omega = omega.astype(np.float64)

    p = 1.0 / (1.0 + np.exp(-delta)) / (1.0 + np.exp(-alpha))   # [D, N]
    q = np.clip(1.0 - p, 1e-30, 1.0)
    w = p * beta * gamma * SCALE                                # [D, N]

    j = np.arange(C)
    qj = np.exp(np.log(q)[:, :, None] * j[None, None, :])       # [D,N,C] q^j
    k = np.einsum('dn,dnj->dj', w, qj)                          # [D, C]
    kp = k.copy()
    kp[:, 0] += omega                                           # fold residual

    kpad = np.zeros((D, 2 * C - 1), np.float64)
    kpad[:, C - 1:] = kp
    idx = (C - 1) + (np.arange(C)[None, :] - np.arange(C)[:, None])  # [s,t]
    AT = kpad[:, idx]                                           # [D, s, t]
    G = qj[:, :, ::-1].transpose(0, 2, 1)                       # [D, s, n]
    ATG = np.concatenate([AT, G], axis=2).astype(np.float16)    # [D,C,C+16]

    E = w[:, :, None] * qj * q[:, :, None]                      # w q^{t+1}
    qC = np.exp(C * np.log(q))                                  # [D, N]

    import ml_dtypes
    et = np.zeros((NCORES, 128, NG * C), ml_dtypes.float8_e4m3)
    qcw = np.zeros((NCORES, 128, NG * C), np.float16)
    cmask = (np.arange(C) % 32 != 0).astype(np.float64)         # reset cols
    for core in range(NCORES):
        off = core * DLOC
        Ec = E[off:off + DLOC].reshape(NG, 4, N, C)             # [g,j,n,t]
        Qc = qC[off:off + DLOC].reshape(NG, 4, N)               # [g,j,n]
        for jj in range(4):
            et[core, 32 * jj:32 * jj + N, :] = (
                Ec[:, jj].transpose(1, 0, 2).reshape(N, NG * C))
            qcw[core, 32 * jj:32 * jj + N, :] = (
                Qc[:, jj].transpose(1, 0)[:, :, None] *
                cmask[None, None, :]).reshape(N, NG * C)
    return ATG, et, qcw


def _make_in_maps(x, ATG, et, qcw):
    in_maps = []
    for core in range(NCORES):
        off = core * DLOC
        # [B, DLOC, NCH, C] -> [C(s), DLOC, B, NCH] -> [128, DLOC*B*NCH]
        xr = np.ascontiguousarray(
            x[:, off:off + DLOC, :].reshape(B, DLOC, NCH, C)
            .transpose(3, 1, 0, 2)
        ).astype(np.float16).reshape(C, DLOC * B * NCH)
        # [DLOC, C, C+N] -> per 8-channel block [C(s), 8, C+N] contiguous
        atg = np.ascontiguousarray(
            ATG[off:off + DLOC].reshape(DLOC // 8, 8, C, C + 16)
            .transpose(0, 2, 1, 3)
        ).reshape((DLOC // 8) * C, 8 * (C + 16))
        in_maps.append({
            "xr": xr,
            "atg": atg,
            "et": et[core],
            "qcw": qcw[core],
        })
    return in_maps


def _gather(results):
    out = np.empty((B, D, L), np.float32)
    for core in range(NCORES):
        off = core * DLOC
        arr = results[core]["out"]                   # [B*NCH, 16, 8, C] f16
        arr = arr.reshape(B, NCH, DLOC // 8, 8, C)
        out[:, off:off + DLOC, :] = (
            arr.transpose(0, 2, 3, 1, 4)
            .reshape(B, DLOC, L).astype(np.float32))
    return out


def kernel(x, delta, alpha, beta, gamma, omega):
    from concourse.bass_utils import run_bass_kernel_spmd

    # force numpy: inputs may arrive as jax arrays, and host math must not
    # round-trip through the device backend
    x, delta, alpha, beta, gamma, omega = (
        np.asarray(a) for a in (x, delta, alpha, beta, gamma, omega))
    ATG, et, qcw = _prep_params(delta, alpha, beta, gamma, omega)
    in_maps = _make_in_maps(x, ATG, et, qcw)

    if "nc" not in _cache:
        _cache["nc"] = _build_program(repeat=1)
    nc = _cache["nc"]

    res = run_bass_kernel_spmd(nc, in_maps, core_ids=list(range(NCORES)))
    return _gather(res.results)


# revision 54
# speedup vs baseline: 1.0026x; 1.0026x over previous
"""Trainium2 Bass kernel for MEGA MultiHeadEMA-style BaseMovingLayer.

Computes, for x[B, D, L] with per-channel EMA params:
    p = sigmoid(delta)*sigmoid(alpha); q = 1-p
    k[d, l] = sum_n (p*beta*gamma*scale)[d,n] * q[d,n]^l
    out = causal_conv(x, k) + x * omega[:, None]

Strategy: shard D=1024 across 8 cores (128 channels/core = 128 SBUF
partitions). Per channel the causal conv is decomposed into chunks of
C=128: intra-chunk via a per-channel Toeplitz matmul (the omega residual
is folded into the k[0] tap), inter-chunk via the EMA state recurrence
at chunk granularity (one tensor_tensor_scan per 4-channel group, with
segment resets encoded as zeros in the multiplier tensor) plus a rank-N
correction matmul. x is transposed and cast to fp16 on the host so every
device DMA is a plain contiguous copy; all matmul operands are fp16,
accumulation is fp32 in PSUM; the output is stored fp16 in a layout that
gives 2KB contiguous runs per partition and is upcast/reassembled on the
host.
"""
import sys
import os
import numpy as np

sys.path.insert(0, "/opt/trn_rl_repo")

B, D, L, N = 4, 1024, 4096, 16
NCORES = 8
DLOC = D // NCORES          # 128 channels per core
C = 128                     # chunk length
NCH = L // C                # 32 chunks
NG = DLOC // 4              # 32 groups of 4 channels
SCALE = 1.0 / np.sqrt(N)

_cache = {}


def _build_program(repeat=1):
    import concourse.bacc as bacc
    import concourse.tile as tile
    import concourse.mybir as mybir

    f16 = mybir.dt.float16
    f32 = mybir.dt.float32
    nc = bacc.Bacc("TRN2", target_bir_lowering=False, debug=False,
                   num_devices=NCORES)

    xr_d = nc.dram_tensor("xr", [C, DLOC * B * NCH], f16,
                          kind="ExternalInput").ap()
    GW = 16
    atg_d = nc.dram_tensor("atg", [(DLOC // 8) * C, 8 * (C + GW)], f16,
                           kind="ExternalInput").ap()
    f8 = mybir.dt.float8e4
    e_d = nc.dram_tensor("et", [128, NG * C], f8, kind="ExternalInput").ap()
    qc_d = nc.dram_tensor("qcw", [128, NG * C], f16,
                          kind="ExternalInput").ap()
    # fp16 out, laid out so each partition (b, c) writes one contiguous
    # 2KB run per 8-channel block; the host reassembles.
    out_d = nc.dram_tensor("out", [B * NCH, DLOC // 8, 8, C], f16,
                           kind="ExternalOutput").ap()

    with tile.TileContext(nc) as tc:
        with (
            tc.tile_pool(name="xt", bufs=1) as xt_pool,
            tc.tile_pool(name="cst", bufs=1) as cst_pool,
            tc.tile_pool(name="atg", bufs=16) as atg_pool,
            tc.tile_pool(name="spsum", bufs=2, space="PSUM") as s_pool,
            tc.tile_pool(name="opsum", bufs=3, space="PSUM") as o_pool,
            tc.tile_pool(name="h", bufs=4) as h_pool,
            tc.tile_pool(name="osb", bufs=16) as ob_pool,
        ):
            for _rep in range(repeat):
                atg_tiles = {}

                def ensure_atg(m):
                    # one tile covers 8 channels (= groups 2m, 2m+1)
                    if m in atg_tiles or m >= DLOC // 8:
                        return atg_tiles.get(m)
                    t = atg_pool.tile([128, 8 * (C + GW)], f16, tag="atg")
                    nc.sync.dma_start(t[:], atg_d[m * C:(m + 1) * C, :])
                    atg_tiles[m] = t
                    return t

                # Critical path: x chunk 0 + atg0 gate the first S-matmul,
                # qc gates the first scan, e gates the first MM2. x streams
                # on the Activation HWDGE queue, params on the SP queue.
                x_all = xt_pool.tile([128, DLOC * B * NCH], f16, tag="xall")
                e_sb = cst_pool.tile([128, NG * C], f8, tag="e")
                qc_sb = cst_pool.tile([128, NG * C], f16, tag="qc")
                xchunk = 2048
                nxc = DLOC * B * NCH // xchunk

                def load_x(i):
                    nc.sync.dma_start(
                        x_all[:, i * xchunk:(i + 1) * xchunk],
                        xr_d[:, i * xchunk:(i + 1) * xchunk],
                    )

                # All inputs on the SP queue in the exact order compute
                # consumes them; the ACT queue carries only output stores,
                # so the DMA engines drain outputs opportunistically.
                qq = NG * C // 4

                def load_sliver(m):
                    nc.sync.dma_start(qc_sb[:, m * qq:(m + 1) * qq],
                                      qc_d[:, m * qq:(m + 1) * qq])
                    nc.sync.dma_start(e_sb[:, m * qq:(m + 1) * qq],
                                      e_d[:, m * qq:(m + 1) * qq])

                load_x(0)
                ensure_atg(0)
                load_sliver(0)
                ensure_atg(1)
                load_x(1)
                ensure_atg(2)
                ensure_atg(3)
                load_x(2)
                ensure_atg(4)
                ensure_atg(5)
                load_sliver(1)
                load_x(3)
                ensure_atg(6)
                ensure_atg(7)
                load_x(4)
                ensure_atg(8)
                ensure_atg(9)
                load_sliver(2)
                load_x(5)
                ensure_atg(10)
                ensure_atg(11)
                load_x(6)
                ensure_atg(12)
                ensure_atg(13)
                load_sliver(3)
                load_x(7)
                ensure_atg(14)
                ensure_atg(15)

                h_tiles = {}
                osb_tiles = {}

                def emit_front(g):
                    # S matmuls + chunk-state scan for group g -> H tile
                    atg_t = ensure_atg(g // 2)
                    ensure_atg(g // 2 + 1)
                    s_q = s_pool.tile([128, 128], f32, tag="s")
                    nc.vector.memset(s_q[:], 0.0)
                    for j in range(4):
                        d = 4 * g + j
                        dd = d % 8
                        nc.tensor.matmul(
                            s_q[32 * j:32 * j + 16, :],
                            lhsT=atg_t[:, dd * (C + GW) + C:
                                       dd * (C + GW) + C + GW],
                            rhs=x_all[:, d * 128:(d + 1) * 128],
                            start=True, stop=True,
                            tile_position=(0, 32 * j),
                        )
                    # H[c] = qC*H[c-1] + S[c]; multiplier has zeros at each
                    # b's c=0 so the recurrence resets per batch element.
                    # Output written shifted by one (-> Hprev); the b-boundary
                    # cols (0/32/64/96) are re-zeroed afterwards (H[b,31] is
                    # never used).
                    h_t = h_pool.tile([128, 132], f16, tag="h")
                    nc.vector.tensor_tensor_scan(
                        out=h_t[:, 1:129],
                        data0=qc_sb[:, g * C:(g + 1) * C],
                        data1=s_q[:, :],
                        initial=0.0,
                        op0=mybir.AluOpType.mult,
                        op1=mybir.AluOpType.add,
                    )
                    nc.gpsimd.memset(
                        h_t[:, 0:128].rearrange("p (b c) -> p b c",
                                                c=32)[:, :, 0:1], 0.0)
                    h_tiles[g] = h_t

                def emit_back(g):
                    # output matmuls + evacuation + store for group g
                    atg_t = ensure_atg(g // 2)
                    h_t = h_tiles.pop(g)
                    blk = g // 2
                    if g % 2 == 0:
                        osb_new = ob_pool.tile([128, 8 * C], f16, tag="osb")
                        osb_tiles[blk] = osb_new
                    osb = osb_tiles[blk]
                    for pair in range(2):
                        o_p = o_pool.tile([128, 1024], f32, tag="o")
                        for j2 in range(2):
                            j = 2 * pair + j2
                            d = 4 * g + j
                            dd = d % 8
                            col0 = j2 * 512
                            nc.tensor.matmul(
                                o_p[:, col0:col0 + 128],
                                lhsT=x_all[:, d * 128:(d + 1) * 128],
                                rhs=atg_t[:, dd * (C + GW):dd * (C + GW) + C],
                                start=True, stop=False,
                            )
                            nc.tensor.matmul(
                                o_p[:, col0:col0 + 128],
                                lhsT=h_t[32 * j:32 * j + 16, 0:128],
                                rhs=e_sb[32 * j:32 * j + 16,
                                         g * C:(g + 1) * C],
                                start=False, stop=True,
                                tile_position=(32 * j, 0),
                            )
                        o_v = o_p[:].rearrange("p (dd z) -> p dd z",
                                               z=512)[:, :, 0:128]
                        dst = osb[:, ((g % 2) * 2 + pair) * 256:
                                  ((g % 2) * 2 + pair + 1) * 256]
                        if pair == 0:
                            nc.vector.tensor_copy(dst, o_v)
                        else:
                            nc.scalar.copy(dst, o_v)
                    if g % 2 == 1:
                        nc.scalar.dma_start(
                            out=out_d[:, blk, :, :],
                            in_=osb[:].rearrange("p (dd t) -> p dd t", t=C),
                        )
                        del osb_tiles[blk]

                DEPTH = 2
                for g in range(NG + DEPTH):
                    if g < NG:
                        emit_front(g)
                    if g >= DEPTH:
                        emit_back(g - DEPTH)

    nc.compile()
    return nc


def _prep_params(delta, alpha, beta, gamma, omega):
    """Host-side derivation of the small conv-kernel operand tensors."""
    delta = delta[..., 0].astype(np.float64)
    alpha = alpha[..., 0].astype(np.float64)
    beta = beta[..., 0].astype(np.float64)
    gamma = gamma.astype(np.float64)
    omega = omega.astype(np.float64)

    p = 1.0 / (1.0 + np.exp(-delta)) / (1.0 + np.exp(-alpha))   # [D, N]
    q = np.clip(1.0 - p, 1e-30, 1.0)
    w = p * beta * gamma * SCALE                                # [D, N]

    j = np.arange(C)
    qj = np.exp(np.log(q)[:, :, None] * j[None, None, :])       # [D,N,C] q^j
    k = np.einsum('dn,dnj->dj', w, qj)                          # [D, C]
    kp = k.copy()
    kp[:, 0] += omega                                           # fold residual

    kpad = np.zeros((D, 2 * C - 1), np.float64)
    kpad[:, C - 1:] = kp
    idx = (C - 1) + (np.arange(C)[None, :] - np.arange(C)[:, None])  # [s,t]
    AT = kpad[:, idx]                                           # [D, s, t]
    G = qj[:, :, ::-1].transpose(0, 2, 1)                       # [D, s, n]
    ATG = np.concatenate([AT, G], axis=2).astype(np.float16)    # [D,C,C+16]

    E = w[:, :, None] * qj * q[:, :, None]                      # w q^{t+1}
    qC = np.exp(C * np.log(q))                                  # [D, N]

    import ml_dtypes
    et = np.zeros((NCORES, 128, NG * C), ml_dtypes.float8_e4m3)
    qcw = np.zeros((NCORES, 128, NG * C), np.float16)
    cmask = (np.arange(C) % 32 != 0).astype(np.float64)         # reset cols
    for core in range(NCORES):
        off = core * DLOC
        Ec = E[off:off + DLOC].reshape(NG, 4, N, C)             # [g,j,n,t]
        Qc = qC[off:off + DLOC].reshape(NG, 4, N)               # [g,j,n]
        for jj in range(4):
            et[core, 32 * jj:32 * jj + N, :] = (
                Ec[:, jj].transpose(1, 0, 2).reshape(N, NG * C))
            qcw[core, 32 * jj:32 * jj + N, :] = (
                Qc[:, jj].transpose(1, 0)[:, :, None] *
                cmask[None, None, :]).reshape(N, NG * C)
    return ATG, et, qcw


def _make_in_maps(x, ATG, et, qcw):
    in_maps = []
    for core in range(NCORES):
        off = core * DLOC
        # [B, DLOC, NCH, C] -> [C(s), DLOC, B, NCH] -> [128, DLOC*B*NCH]
        xr = np.ascontiguousarray(
            x[:, off:off + DLOC, :].reshape(B, DLOC, NCH, C)
            .transpose(3, 1, 0, 2)
        ).astype(np.float16).reshape(C, DLOC * B * NCH)
        # [DLOC, C, C+N] -> per 8-channel block [C(s), 8, C+N] contiguous
        atg = np.ascontiguousarray(
            ATG[off:off + DLOC].reshape(DLOC // 8, 8, C, C + 16)
            .transpose(0, 2, 1, 3)
        ).reshape((DLOC // 8) * C, 8 * (C + 16))
        in_maps.append({
            "xr": xr,
            "atg": atg,
            "et": et[core],
            "qcw": qcw[core],
        })
    return in_maps


def _gather(results):
    out = np.empty((B, D, L), np.float32)
    for core in range(NCORES):
        off = core * DLOC
        arr = results[core]["out"]                   # [B*NCH, 16, 8, C] f16
        arr = arr.reshape(B, NCH, DLOC // 8, 8, C)
        out[:, off:off + DLOC, :] = (
            arr.transpose(0, 2, 3, 1, 4)
            .reshape(B, DLOC, L).astype(np.float32))
    return out


def kernel(x, delta, alpha, beta, gamma, omega):
    from concourse.bass_utils import run_bass_kernel_spmd

    # force numpy: inputs may arrive as jax arrays, and host math must not
    # round-trip through the device backend
    x, delta, alpha, beta, gamma, omega = (
        np.asarray(a) for a in (x, delta, alpha, beta, gamma, omega))
    ATG, et, qcw = _prep_params(delta, alpha, beta, gamma, omega)
    in_maps = _make_in_maps(x, ATG, et, qcw)

    if "nc" not in _cache:
        _cache["nc"] = _build_program(repeat=1)
    nc = _cache["nc"]

    res = run_bass_kernel_spmd(nc, in_maps, core_ids=list(range(NCORES)))
    return _gather(res.results)
